# revision 39
# baseline (speedup 1.0000x reference)
"""Trainium2 Bass kernel for nn_CNN_ternary (ternary CNN, 8-core data parallel).

Strategy:
  - All weights/BN folded on host (fp64). Activations after every layer are
    exactly ternary {-1,0,1}; all conv/fc matmuls on integer-exact data.
  - L1 (continuous input): x split into 3 stacked fp16 planes (hi/lo1/lo2),
    single K=108 matmul per output tile reproduces fp32-accurate conv.
  - L2/L3: shift-accumulate convs (taps = free-dim shifts), batch striped
    across partition strips with tile_position concurrency.
  - Ternarization: per-channel affine on PSUM via ScalarE activation
    (scale/bias APs) + magic-constant rounding + dual-op tensor_scalar clips.
    Integer thresholds host-verified exhaustively (bit-exact vs reference).
"""

import sys

sys.path.insert(0, "/opt/trn_rl_repo")

import numpy as np
import ml_dtypes

DELTA = 0.1
BN_EPS = 1e-5
M32 = 12582912.0  # 1.5 * 2^23  fp32 round-to-int magic
N_CORES = 8
B_FULL = 4096
BC = B_FULL // N_CORES  # 512 per core
BT = 64                 # batch tile
NBT = BC // BT          # 8
# per b-tile: 4 strips (b-hat) x 16 bq
NSTRIP = 4
NBQ = BT // NSTRIP      # 16


def _tern(t):
    return np.where(np.abs(t) < DELTA, 0.0, np.sign(t))


def _bf16(x):
    return np.asarray(x, dtype=np.float32).astype(ml_dtypes.bfloat16)


def _affine(i, d):
    """Return (A, B) fp64 such that z_i = A*conv + B, conv using tern weights."""
    g = d[f"g{i}"].astype(np.float64)
    be = d[f"be{i}"].astype(np.float64)
    m = d[f"m{i}"].astype(np.float64)
    v = d[f"v{i}"].astype(np.float64)
    tb = _tern(d[f"b{i}"].astype(np.float64))
    A = g / np.sqrt(v + BN_EPS)
    B = (tb - m) * A + be
    return A, B


def _int_threshold_map(A, B, pmax):
    """Integer-psum ternarization: act computes w=p*alpha+beta (fp32), bf16(w)
    then ts chain (max,191)(sub,191);(min,2)(sub,1) must yield exact tern.
    Host-verifies exhaustively over p in [-pmax,pmax] under both single-fma
    and mul-then-add rounding semantics; nudges beta on failure."""
    A = A.copy()
    sg = np.where(A < 0, -1.0, 1.0)
    Aa = np.abs(A)
    thp = (DELTA - B) / Aa
    thm = (-DELTA - B) / Aa
    Kp = np.ceil(thp)
    Kp = np.where(Kp < thp, Kp + 1, Kp)  # paranoia
    Km = np.floor(thm)
    Km = np.where(Km > thm, Km - 1, Km)
    G = Kp - Km
    assert np.all(G >= 1)
    alpha = 2.0 / (2.0 * G - 1.0)
    beta = 192.0 - (Kp + Km) / 2.0 * alpha
    a32 = alpha.astype(np.float32)
    b32 = beta.astype(np.float32)

    p = np.arange(-pmax, pmax + 1, dtype=np.float64)
    true_t = (p[None, :] >= Kp[:, None]).astype(np.int32) - (
        p[None, :] <= Km[:, None]
    ).astype(np.int32)

    def decisions(a32v, b32v):
        outs = []
        for mode in range(2):
            if mode == 0:  # single-rounding fma
                w = (p[None, :] * a32v[:, None].astype(np.float64)
                     + b32v[:, None].astype(np.float64)).astype(np.float32)
            else:  # mul then add, both fp32-rounded
                w = (p[None, :].astype(np.float32) * a32v[:, None]).astype(
                    np.float32) + b32v[:, None]
            wb = w.astype(ml_dtypes.bfloat16).astype(np.float32)
            # kernel chain: p1 = bf16(max(wb,191) - 192); p2 = bf16(min(p1,1))
            y1 = np.maximum(wb, np.float32(191.0)) - np.float32(192.0)
            y1 = y1.astype(ml_dtypes.bfloat16).astype(np.float32)
            y2 = np.minimum(y1, np.float32(1.0))
            y2 = y2.astype(ml_dtypes.bfloat16).astype(np.float32)
            outs.append(y2.astype(np.int32))
        return outs

    for _ in range(40):
        d0, d1 = decisions(a32, b32)
        bad = np.any(d0 != true_t, axis=1) | np.any(d1 != true_t, axis=1)
        if not bad.any():
            break
        # nudge beta slightly for failing channels
        for c in np.where(bad)[0]:
            b32[c] = np.float32(b32[c] + 1e-4 * a32[c] * (1 if (_ % 2 == 0) else -1) * (_ // 2 + 1))
    else:
        raise RuntimeError("threshold map verification failed")
    return sg, a32, b32, Km.astype(np.float32)


def _build_host_tensors(inputs):
    d = inputs
    # --- layer params ---
    A1, B1 = _affine(1, d)
    A2, B2 = _affine(2, d)
    A3, B3 = _affine(3, d)
    A4, B4 = _affine(4, d)

    sg1 = np.where(A1 < 0, -1.0, 1.0)
    al1 = (np.abs(A1) / (2 * DELTA)).astype(np.float32)       # [32]
    # center at 192 so the ACT's bf16 output rounds to integers (same
    # magic as L2-4); ternarize chain is then min(max(w,191)-192, 1).
    be1 = (B1 / (2 * DELTA) + 192.0).astype(np.float32)       # [32]

    sg2, al2, be2, km2 = _int_threshold_map(A2, B2, pmax=100)     # [64]
    sg3, al3, be3, km3 = _int_threshold_map(A3, B3, pmax=200)     # [128]
    sg4, al4, be4, km4 = _int_threshold_map(A4, B4, pmax=800)     # [128]

    w1t = _tern(d["w1"].astype(np.float64))[:, 0, 0, :] * sg1[:, None]   # [32,9]
    w2t = _tern(d["w2"].astype(np.float64)) * sg2[:, None, None, None]   # [64,32,1,3]
    w3t = _tern(d["w3"].astype(np.float64)) * sg3[:, None, None, None]   # [128,64,1,3]
    w4t = _tern(d["w4"].astype(np.float64)) * sg4[:, None, None, None]   # [128,128,6,1]
    wft = _tern(d["wf"].astype(np.float64))                              # [10,2048]
    tbf = _tern(d["bf"].astype(np.float64)).astype(np.float32)           # [10]

    # --- lhsT tensors ---
    # T1 [108,128] fp16: row = s*36 + bh*9 + t ; col = bh*32 + c
    T1 = np.zeros((108, 128), np.float64)
    for bh in range(4):
        for t in range(9):
            for s in range(3):
                T1[s * 36 + bh * 9 + t, bh * 32:bh * 32 + 32] = w1t[:, t]
    T1 = T1.astype(np.float16)

    # W2 [128,3,64] bf16: rows 32g+ci -> w2t[c, ci, 0, t]
    W2 = np.zeros((128, 3, 64), np.float64)
    for g in range(4):
        W2[32 * g:32 * g + 32] = w2t[:, :, 0, :].transpose(1, 2, 0)  # [ci,t,c]
    W2 = _bf16(W2)

    # W3 [128,3,128]: rows 64ch+ci -> w3t[c, ci, 0, t]
    W3 = np.zeros((128, 3, 128), np.float64)
    for ch in range(2):
        W3[64 * ch:64 * ch + 64] = w3t[:, :, 0, :].transpose(1, 2, 0)
    W3 = _bf16(W3)

    # W4 [128,6,128]: rows ci -> w4t[c, ci, h, 0]
    W4 = _bf16(w4t[:, :, :, 0].transpose(1, 2, 0))  # [ci,h,c]

    # WF [128,16,10]: [c, w, o] = wft[o, c*16+w]
    WF = _bf16(wft.reshape(10, 128, 16).transpose(1, 2, 0))

    # --- act vectors [128,1] fp32 ---
    AL1 = np.tile(al1, 4)[:, None].astype(np.float32)
    BE1 = np.tile(be1, 4)[:, None].astype(np.float32)
    AL2 = np.tile(al2, 2)[:, None].astype(np.float32)
    BE2 = np.tile(be2, 2)[:, None].astype(np.float32)
    AL3 = al3[:, None].astype(np.float32)
    BE3 = be3[:, None].astype(np.float32)
    AL4 = al4[:, None].astype(np.float32)
    BE4 = be4[:, None].astype(np.float32)
    KM3 = km3[:, None].astype(np.float32)
    NB1 = np.full((128, 1), -191.0, np.float32)
    # L1 pool clamp in raw-psum space: maps to w = 191.25 (below the 191.5
    # decision boundary), so clamped values ternarize to -1 exactly.
    thm1_ = (-DELTA - B1) / np.abs(A1)
    TH1 = np.tile(thm1_ - 0.25 / (np.abs(A1) / (2 * DELTA)),
                  4)[:, None].astype(np.float32)
    TBF = np.zeros((16, 1), np.float32)
    TBF[:10, 0] = tbf

    consts = dict(T1=T1, W2=W2, W3=W3, W4=W4, WF=WF,
                  AL1=AL1, BE1=BE1, AL2=AL2, BE2=BE2, AL3=AL3, BE3=BE3,
                  AL4=AL4, BE4=BE4, KM3=KM3, TH1=TH1, NB1=NB1, TBF=TBF)

    # --- X1 im2col per core: [108, NBT*NBQ*6*64] fp16 ---
    x = d["x"].astype(np.float32)[:, 0]          # [4096, 6, 128]
    xp = np.pad(x, ((0, 0), (0, 0), (4, 4)))     # [4096, 6, 136]
    x0 = xp.astype(np.float16)
    r1 = (xp - x0.astype(np.float32))
    x1 = r1.astype(np.float16)
    x2f = (r1 - x1.astype(np.float32))           # fp32 working copy of lo2

    # window gather: for j in [0,64): q = 2j + t, t in [0,9)
    j = np.arange(64)
    t = np.arange(9)
    qidx = (2 * j[None, :] + t[:, None])         # [9, 64]

    # --- margin guard: protect near-threshold L1 elements against fp32
    # accumulation-order noise by nudging the lo2 plane away from thresholds.
    thp1 = (DELTA - B1) / np.abs(A1)             # p-space thresholds [32]
    thm1 = (-DELTA - B1) / np.abs(A1)
    # margin covers fp32 accumulation noise (~1.5e-5) plus the fp32
    # mul/add rounding of the w=al*p+be+192 affine (~4.5e-5 in w units,
    # /al>=2.8 in p units)
    TOL = 4.5e-5
    w64 = w1t.astype(np.float64)
    xs64 = (x0.astype(np.float64) + x1.astype(np.float64) + x2f.astype(np.float64))
    for _pass in range(3):
        nfix = 0
        for b0 in range(0, B_FULL, 512):
            blk = xs64[b0:b0 + 512]
            pe = np.einsum('bhtj,ct->bchj', blk[:, :, qidx], w64)
            for thr in (thp1, thm1):
                dist = pe - thr[None, :, None, None]
                bad = np.argwhere(np.abs(dist) < TOL)
                for bb, cc, hh, jj in bad:
                    dv = dist[bb, cc, hh, jj]
                    dp = np.sign(dv) * (2.0 * TOL - abs(dv)) if dv != 0 else 2.0 * TOL
                    for tt in range(9):
                        q = 2 * jj + tt
                        if w64[cc, tt] != 0 and 4 <= q < 132:
                            x2f[b0 + bb, hh, q] += np.float32(dp / w64[cc, tt])
                            xs64[b0 + bb, hh, q] = (x0[b0 + bb, hh, q].astype(np.float64)
                                                    + x1[b0 + bb, hh, q].astype(np.float64)
                                                    + np.float64(np.float16(x2f[b0 + bb, hh, q])))
                            nfix += 1
                            break
        if nfix == 0:
            break
    x2 = x2f.astype(np.float16)
    splits = [x0, x1, x2]

    X1s = []
    for cr in range(N_CORES):
        X1 = np.empty((108, NBT * NBQ * 6 * 64), np.float16)
        for s in range(3):
            xs = splits[s]
            for bh in range(4):
                # b global = cr*BC + bt*BT + bh*NBQ + bq
                bidx = (cr * BC + np.arange(NBT)[:, None] * BT + bh * NBQ
                        + np.arange(NBQ)[None, :]).reshape(-1)   # [NBT*NBQ]
                blk = xs[bidx][:, :, qidx]                       # [nb, 6, 9, 64]
                blk = blk.transpose(2, 0, 1, 3)                  # [9, nb, 6, 64]
                X1[s * 36 + bh * 9: s * 36 + bh * 9 + 9] = blk.reshape(9, -1)
        X1s.append(X1)
    return consts, X1s


def _build_program():
    import concourse.bass as bass
    import concourse.tile as tile
    from concourse import bacc, mybir

    F = mybir.dt.float32
    H = mybir.dt.float16
    BF = mybir.dt.bfloat16
    AO = mybir.AluOpType
    ACT = mybir.ActivationFunctionType.Identity
    RELU = mybir.ActivationFunctionType.Relu

    nc = bacc.Bacc("TRN2", target_bir_lowering=False)

    NCOL1 = NBT * NBQ * 6 * 64
    X1 = nc.dram_tensor("X1", [108, NCOL1], H, kind="ExternalInput")
    T1 = nc.dram_tensor("T1", [108, 128], H, kind="ExternalInput")
    W2 = nc.dram_tensor("W2", [128, 3, 64], BF, kind="ExternalInput")
    W3 = nc.dram_tensor("W3", [128, 3, 128], BF, kind="ExternalInput")
    W4 = nc.dram_tensor("W4", [128, 6, 128], BF, kind="ExternalInput")
    WF = nc.dram_tensor("WF", [128, 16, 10], BF, kind="ExternalInput")
    vecs = {}
    for nm in ["AL1", "BE1", "AL2", "BE2", "AL3", "BE3", "AL4", "BE4",
               "KM3", "TH1", "NB1"]:
        vecs[nm] = nc.dram_tensor(nm, [128, 1], F, kind="ExternalInput")
    TBF = nc.dram_tensor("TBF", [16, 1], F, kind="ExternalInput")
    OUT = nc.dram_tensor("OUT", [BC, 10], F, kind="ExternalOutput")

    from contextlib import ExitStack
    with tile.TileContext(nc) as tc, ExitStack() as es:
        wp = es.enter_context(tc.tile_pool(name="wp", bufs=1))
        xp_ = es.enter_context(tc.tile_pool(name="xp", bufs=2))
        ap1 = es.enter_context(tc.tile_pool(name="ap1", bufs=2))
        ap2 = es.enter_context(tc.tile_pool(name="ap2", bufs=2))
        ap3 = es.enter_context(tc.tile_pool(name="ap3", bufs=2))
        ap4 = es.enter_context(tc.tile_pool(name="ap4", bufs=2))
        stg = es.enter_context(tc.tile_pool(name="stg", bufs=4))
        # one psum pool: chunks of 4 full banks, double-buffered = all 8 banks
        pcp = es.enter_context(tc.tile_pool(name="pcp", bufs=2, space="PSUM"))

        t1t = wp.tile([108, 128], H)
        nc.sync.dma_start(t1t[:], T1[:])
        w2t = wp.tile([128, 3, 64], BF)
        nc.sync.dma_start(w2t[:], W2[:])
        w3t = wp.tile([128, 3, 128], BF)
        nc.sync.dma_start(w3t[:], W3[:])
        w4t = wp.tile([128, 6, 128], BF)
        nc.sync.dma_start(w4t[:], W4[:])
        wft = wp.tile([128, 16, 10], BF)
        nc.sync.dma_start(wft[:], WF[:])
        vt = {}
        for nm, dr in vecs.items():
            vt[nm] = wp.tile([128, 1], F, tag=nm, name=nm.lower())
            nc.sync.dma_start(vt[nm][:], dr[:])
        tbft = wp.tile([16, 1], F)
        nc.sync.dma_start(tbft[:], TBF[:])

        for bt in range(NBT):
            # ---------- L1 ----------
            x1t = xp_.tile([108, NBQ * 6 * 64], H, tag="x1")
            nc.sync.dma_start(
                x1t[:], X1[:, bt * NBQ * 6 * 64:(bt + 1) * NBQ * 6 * 64])
            a1 = ap1.tile([128, NBQ, 6, 34], BF, tag="a1")
            nc.vector.memset(a1[:, :, :, 0:1], 0.0)
            nc.vector.memset(a1[:, :, :, 33:34], 0.0)
            for c in (0, 2, 1, 3):
                pc = pcp.tile([128, 4, 512], F, tag="pc")
                for b in range(4):
                    bq = c * 4 + b
                    nc.tensor.matmul(pc[:, b, 0:384], t1t[:],
                                     x1t[:, bq * 384:(bq + 1) * 384],
                                     start=True, stop=True)
                w1c = stg.tile([128, 4, 384], BF, tag="w1c")
                nc.scalar.activation(w1c[:], pc[:, :, 0:384], ACT,
                                     bias=vt["BE1"][:], scale=vt["AL1"][:])
                wv = w1c[:].rearrange("p c (h v e) -> p c h v e", v=32, e=2)
                y1 = stg.tile([128, 4, 6, 32], BF, tag="y1")
                nc.vector.scalar_tensor_tensor(
                    y1[:], wv[:, :, :, :, 0], 191.0, wv[:, :, :, :, 1],
                    AO.max, AO.max)
                nc.vector.tensor_scalar(a1[:, c * 4:c * 4 + 4, :, 1:33],
                                        y1[:], 192.0, 1.0,
                                        AO.subtract, AO.min)
            # ---------- L2 ----------
            # Issue order interleaves the 4 row strips (g) so their rhs
            # streams run concurrently on disjoint 32-row PE strips.
            a2 = ap2.tile([128, 4, 8, 6, 34], BF, tag="a2")
            nc.vector.memset(a2[:, :, :, :, 0:1], 0.0)
            nc.vector.memset(a2[:, :, :, :, 33:34], 0.0)
            for ck in range(4):
                pc = pcp.tile([128, 4, 512], F, tag="pc")
                for t in range(3):
                    for ch in range(2):
                        bq0 = ch * 8 + ck * 2
                        for g in range(4):
                            nc.tensor.matmul(
                                pc[64 * ch:64 * ch + 64, g, 0:384],
                                w2t[32 * g:32 * g + 32, t, :],
                                a1[32 * g:32 * g + 32, bq0:bq0 + 2, :, t:t + 32],
                                start=(t == 0), stop=(t == 2),
                                tile_position=(32 * g, 64 * ch))
                w2c = stg.tile([128, 4, 384], BF, tag="w2c")
                nc.scalar.activation(w2c[:], pc[:, :, 0:384], ACT,
                                     bias=vt["BE2"][:], scale=vt["AL2"][:])
                y2 = stg.tile([128, 4, 384], BF, tag="y2")
                nc.vector.tensor_scalar(y2[:], w2c[:], 191.0, 192.0,
                                        AO.max, AO.subtract)
                nc.vector.tensor_scalar(
                    a2[:, :, ck * 2:ck * 2 + 2, :, 1:33],
                    y2[:].rearrange("p c (b h v) -> p c b h v", b=2, h=6),
                    1.0, None, AO.min)
            # ---------- L3 ----------
            # Interleave the two 64-row strips (ch = batch half) so both
            # streams run concurrently; psum tags shared with L2's pool.
            a3 = ap3.tile([128, 4, 16, 6, 16], BF, tag="a3")
            for g in range(4):
                for j in range(2):
                    pc = pcp.tile([128, 4, 512], F, tag="pc")
                    for t in range(3):
                        for ch in range(2):
                            for bpp in range(2):
                                bp = 2 * j + bpp
                                nc.tensor.matmul(
                                    pc[:, 2 * ch + bpp, 0:384],
                                    w3t[64 * ch:64 * ch + 64, t, :],
                                    a2[64 * ch:64 * ch + 64, g,
                                       bp * 2:bp * 2 + 2, :, t:t + 32],
                                    start=(t == 0), stop=(t == 2),
                                    tile_position=(64 * ch, 0))
                    w3c = stg.tile([128, 4, 384], BF, tag="w3c")
                    nc.scalar.activation(w3c[:], pc[:, :, 0:384], ACT,
                                         bias=vt["BE3"][:],
                                         scale=vt["AL3"][:])
                    wv3 = w3c[:].rearrange("p c (b h v e) -> p c b h v e",
                                           b=2, h=6, e=2)
                    for ch in range(2):
                        y3 = stg.tile([128, 2, 2, 6, 16], BF, tag="y3")
                        nc.vector.scalar_tensor_tensor(
                            y3[:], wv3[:, 2 * ch:2 * ch + 2, :, :, :, 0],
                            191.0, wv3[:, 2 * ch:2 * ch + 2, :, :, :, 1],
                            AO.max, AO.max)
                        nc.vector.tensor_scalar(
                            a3[:, g, 8 * ch + 4 * j:8 * ch + 4 * j + 4, :, :],
                            y3[:].rearrange("p q b h v -> p (q b) h v"),
                            192.0, 1.0, AO.subtract, AO.min)
            # ---------- L4 + FC (one psum chunk: banks 0,1 conv, bank 3 fc) --
            a4 = ap4.tile([128, 4, 16, 16], BF, tag="a4")
            pc = pcp.tile([128, 4, 512], F, tag="pc")
            for h in range(6):
                for ck in range(2):
                    nc.tensor.matmul(pc[:, ck, 0:512], w4t[:, h, :],
                                     a3[:, ck * 2:ck * 2 + 2, :, h, :],
                                     start=(h == 0), stop=(h == 5))
            w4c = stg.tile([128, 2, 512], BF, tag="w4c")
            nc.vector.tensor_scalar(w4c[:], pc[:, 0:2, 0:512],
                                    vt["AL4"][:], vt["BE4"][:],
                                    AO.mult, AO.add)
            y4 = stg.tile([128, 2, 512], BF, tag="y4")
            nc.vector.tensor_scalar(y4[:], w4c[:], 191.0, 192.0,
                                    AO.max, AO.subtract)
            nc.vector.tensor_scalar(
                a4[:],
                y4[:].rearrange("p c (g b v) -> p (c g) b v", g=2, b=16),
                1.0, None, AO.min)
            for w in range(16):
                nc.tensor.matmul(pc[0:10, 3, 0:64], wft[:, w, :],
                                 a4[:, :, :, w],
                                 start=(w == 0), stop=(w == 15))
            fo = stg.tile([16, 64], F, tag="fo")
            nc.scalar.activation(fo[0:10, :], pc[0:10, 3, 0:64], ACT,
                                 bias=tbft[0:10, :], scale=1.0)
            nc.sync.dma_start(
                OUT[bt * BT:(bt + 1) * BT, :].rearrange("b o -> o b"),
                fo[0:10, :])

    nc.finalize()
    return nc


_CACHED = {}


def kernel(**inputs):
    from concourse.bass_utils import run_bass_kernel_spmd

    consts, X1s = _build_host_tensors(inputs)
    if "nc" not in _CACHED:
        _CACHED["nc"] = _build_program()
    nc = _CACHED["nc"]

    in_maps = []
    for cr in range(N_CORES):
        m = {k: np.ascontiguousarray(v) for k, v in consts.items()}
        m["X1"] = np.ascontiguousarray(X1s[cr])
        in_maps.append(m)

    res = run_bass_kernel_spmd(nc, in_maps, list(range(N_CORES)))
    out = np.concatenate([res.results[cr]["OUT"] for cr in range(N_CORES)], 0)
    return out.astype(np.float32)



# revision 40
# speedup vs baseline: 1.0085x; 1.0085x over previous
"""Trainium2 Bass kernel for nn_CNN_ternary (ternary CNN, 8-core data parallel).

Strategy:
  - All weights/BN folded on host (fp64). Activations after every layer are
    exactly ternary {-1,0,1}; all conv/fc matmuls on integer-exact data.
  - L1 (continuous input): x split into 3 stacked fp16 planes (hi/lo1/lo2),
    single K=108 matmul per output tile reproduces fp32-accurate conv.
  - L2/L3: shift-accumulate convs (taps = free-dim shifts), batch striped
    across partition strips with tile_position concurrency.
  - Ternarization: per-channel affine on PSUM via ScalarE activation
    (scale/bias APs) + magic-constant rounding + dual-op tensor_scalar clips.
    Integer thresholds host-verified exhaustively (bit-exact vs reference).
"""

import sys

sys.path.insert(0, "/opt/trn_rl_repo")

import numpy as np
import ml_dtypes

DELTA = 0.1
BN_EPS = 1e-5
M32 = 12582912.0  # 1.5 * 2^23  fp32 round-to-int magic
N_CORES = 8
B_FULL = 4096
BC = B_FULL // N_CORES  # 512 per core
BT = 64                 # batch tile
NBT = BC // BT          # 8
# per b-tile: 4 strips (b-hat) x 16 bq
NSTRIP = 4
NBQ = BT // NSTRIP      # 16


def _tern(t):
    return np.where(np.abs(t) < DELTA, 0.0, np.sign(t))


def _bf16(x):
    return np.asarray(x, dtype=np.float32).astype(ml_dtypes.bfloat16)


def _affine(i, d):
    """Return (A, B) fp64 such that z_i = A*conv + B, conv using tern weights."""
    g = d[f"g{i}"].astype(np.float64)
    be = d[f"be{i}"].astype(np.float64)
    m = d[f"m{i}"].astype(np.float64)
    v = d[f"v{i}"].astype(np.float64)
    tb = _tern(d[f"b{i}"].astype(np.float64))
    A = g / np.sqrt(v + BN_EPS)
    B = (tb - m) * A + be
    return A, B


def _int_threshold_map(A, B, pmax):
    """Integer-psum ternarization: act computes w=p*alpha+beta (fp32), bf16(w)
    then ts chain (max,191)(sub,191);(min,2)(sub,1) must yield exact tern.
    Host-verifies exhaustively over p in [-pmax,pmax] under both single-fma
    and mul-then-add rounding semantics; nudges beta on failure."""
    A = A.copy()
    sg = np.where(A < 0, -1.0, 1.0)
    Aa = np.abs(A)
    thp = (DELTA - B) / Aa
    thm = (-DELTA - B) / Aa
    Kp = np.ceil(thp)
    Kp = np.where(Kp < thp, Kp + 1, Kp)  # paranoia
    Km = np.floor(thm)
    Km = np.where(Km > thm, Km - 1, Km)
    G = Kp - Km
    assert np.all(G >= 1)
    alpha = 2.0 / (2.0 * G - 1.0)
    beta = 192.0 - (Kp + Km) / 2.0 * alpha
    a32 = alpha.astype(np.float32)
    b32 = beta.astype(np.float32)

    p = np.arange(-pmax, pmax + 1, dtype=np.float64)
    true_t = (p[None, :] >= Kp[:, None]).astype(np.int32) - (
        p[None, :] <= Km[:, None]
    ).astype(np.int32)

    def decisions(a32v, b32v):
        outs = []
        for mode in range(2):
            if mode == 0:  # single-rounding fma
                w = (p[None, :] * a32v[:, None].astype(np.float64)
                     + b32v[:, None].astype(np.float64)).astype(np.float32)
            else:  # mul then add, both fp32-rounded
                w = (p[None, :].astype(np.float32) * a32v[:, None]).astype(
                    np.float32) + b32v[:, None]
            wb = w.astype(ml_dtypes.bfloat16).astype(np.float32)
            # kernel chain: p1 = bf16(max(wb,191) - 192); p2 = bf16(min(p1,1))
            y1 = np.maximum(wb, np.float32(191.0)) - np.float32(192.0)
            y1 = y1.astype(ml_dtypes.bfloat16).astype(np.float32)
            y2 = np.minimum(y1, np.float32(1.0))
            y2 = y2.astype(ml_dtypes.bfloat16).astype(np.float32)
            outs.append(y2.astype(np.int32))
        return outs

    for _ in range(40):
        d0, d1 = decisions(a32, b32)
        bad = np.any(d0 != true_t, axis=1) | np.any(d1 != true_t, axis=1)
        if not bad.any():
            break
        # nudge beta slightly for failing channels
        for c in np.where(bad)[0]:
            b32[c] = np.float32(b32[c] + 1e-4 * a32[c] * (1 if (_ % 2 == 0) else -1) * (_ // 2 + 1))
    else:
        raise RuntimeError("threshold map verification failed")
    return sg, a32, b32, Km.astype(np.float32)


def _build_host_tensors(inputs):
    d = inputs
    # --- layer params ---
    A1, B1 = _affine(1, d)
    A2, B2 = _affine(2, d)
    A3, B3 = _affine(3, d)
    A4, B4 = _affine(4, d)

    sg1 = np.where(A1 < 0, -1.0, 1.0)
    al1 = (np.abs(A1) / (2 * DELTA)).astype(np.float32)       # [32]
    # center at 192 so the ACT's bf16 output rounds to integers (same
    # magic as L2-4); ternarize chain is then min(max(w,191)-192, 1).
    be1 = (B1 / (2 * DELTA) + 192.0).astype(np.float32)       # [32]

    sg2, al2, be2, km2 = _int_threshold_map(A2, B2, pmax=100)     # [64]
    sg3, al3, be3, km3 = _int_threshold_map(A3, B3, pmax=200)     # [128]
    sg4, al4, be4, km4 = _int_threshold_map(A4, B4, pmax=800)     # [128]

    w1t = _tern(d["w1"].astype(np.float64))[:, 0, 0, :] * sg1[:, None]   # [32,9]
    w2t = _tern(d["w2"].astype(np.float64)) * sg2[:, None, None, None]   # [64,32,1,3]
    w3t = _tern(d["w3"].astype(np.float64)) * sg3[:, None, None, None]   # [128,64,1,3]
    w4t = _tern(d["w4"].astype(np.float64)) * sg4[:, None, None, None]   # [128,128,6,1]
    wft = _tern(d["wf"].astype(np.float64))                              # [10,2048]
    tbf = _tern(d["bf"].astype(np.float64)).astype(np.float32)           # [10]

    # --- lhsT tensors ---
    # T1 [108,128] fp16: row = s*36 + bh*9 + t ; col = bh*32 + c
    T1 = np.zeros((108, 128), np.float64)
    for bh in range(4):
        for t in range(9):
            for s in range(3):
                T1[s * 36 + bh * 9 + t, bh * 32:bh * 32 + 32] = w1t[:, t]
    T1 = T1.astype(np.float16)

    # W2 [128,3,64] bf16: rows 32g+ci -> w2t[c, ci, 0, t]
    W2 = np.zeros((128, 3, 64), np.float64)
    for g in range(4):
        W2[32 * g:32 * g + 32] = w2t[:, :, 0, :].transpose(1, 2, 0)  # [ci,t,c]
    W2 = _bf16(W2)

    # W3 [128,3,128]: rows 64ch+ci -> w3t[c, ci, 0, t]
    W3 = np.zeros((128, 3, 128), np.float64)
    for ch in range(2):
        W3[64 * ch:64 * ch + 64] = w3t[:, :, 0, :].transpose(1, 2, 0)
    W3 = _bf16(W3)

    # W4 [128,6,128]: rows ci -> w4t[c, ci, h, 0]
    W4 = _bf16(w4t[:, :, :, 0].transpose(1, 2, 0))  # [ci,h,c]

    # WF [128,16,10]: [c, w, o] = wft[o, c*16+w]
    WF = _bf16(wft.reshape(10, 128, 16).transpose(1, 2, 0))

    # --- act vectors [128,1] fp32 ---
    AL1 = np.tile(al1, 4)[:, None].astype(np.float32)
    BE1 = np.tile(be1, 4)[:, None].astype(np.float32)
    AL2 = np.tile(al2, 2)[:, None].astype(np.float32)
    BE2 = np.tile(be2, 2)[:, None].astype(np.float32)
    AL3 = al3[:, None].astype(np.float32)
    BE3 = be3[:, None].astype(np.float32)
    AL4 = al4[:, None].astype(np.float32)
    BE4 = be4[:, None].astype(np.float32)
    KM3 = km3[:, None].astype(np.float32)
    NB1 = np.full((128, 1), -191.0, np.float32)
    # L1 pool clamp in raw-psum space: maps to w = 191.25 (below the 191.5
    # decision boundary), so clamped values ternarize to -1 exactly.
    thm1_ = (-DELTA - B1) / np.abs(A1)
    TH1 = np.tile(thm1_ - 0.25 / (np.abs(A1) / (2 * DELTA)),
                  4)[:, None].astype(np.float32)
    TBF = np.zeros((16, 1), np.float32)
    TBF[:10, 0] = tbf

    consts = dict(T1=T1, W2=W2, W3=W3, W4=W4, WF=WF,
                  AL1=AL1, BE1=BE1, AL2=AL2, BE2=BE2, AL3=AL3, BE3=BE3,
                  AL4=AL4, BE4=BE4, KM3=KM3, TH1=TH1, NB1=NB1, TBF=TBF)

    # --- X1 im2col per core: [108, NBT*NBQ*6*64] fp16 ---
    x = d["x"].astype(np.float32)[:, 0]          # [4096, 6, 128]
    xp = np.pad(x, ((0, 0), (0, 0), (4, 4)))     # [4096, 6, 136]
    x0 = xp.astype(np.float16)
    r1 = (xp - x0.astype(np.float32))
    x1 = r1.astype(np.float16)
    x2f = (r1 - x1.astype(np.float32))           # fp32 working copy of lo2

    # window gather: for j in [0,64): q = 2j + t, t in [0,9)
    j = np.arange(64)
    t = np.arange(9)
    qidx = (2 * j[None, :] + t[:, None])         # [9, 64]

    # --- margin guard: protect near-threshold L1 elements against fp32
    # accumulation-order noise by nudging the lo2 plane away from thresholds.
    thp1 = (DELTA - B1) / np.abs(A1)             # p-space thresholds [32]
    thm1 = (-DELTA - B1) / np.abs(A1)
    # margin covers fp32 accumulation noise (~1.5e-5) plus the fp32
    # mul/add rounding of the w=al*p+be+192 affine (~4.5e-5 in w units,
    # /al>=2.8 in p units)
    TOL = 4.5e-5
    w64 = w1t.astype(np.float64)
    xs64 = (x0.astype(np.float64) + x1.astype(np.float64) + x2f.astype(np.float64))
    for _pass in range(3):
        nfix = 0
        for b0 in range(0, B_FULL, 512):
            blk = xs64[b0:b0 + 512]
            pe = np.einsum('bhtj,ct->bchj', blk[:, :, qidx], w64)
            for thr in (thp1, thm1):
                dist = pe - thr[None, :, None, None]
                bad = np.argwhere(np.abs(dist) < TOL)
                for bb, cc, hh, jj in bad:
                    dv = dist[bb, cc, hh, jj]
                    dp = np.sign(dv) * (2.0 * TOL - abs(dv)) if dv != 0 else 2.0 * TOL
                    for tt in range(9):
                        q = 2 * jj + tt
                        if w64[cc, tt] != 0 and 4 <= q < 132:
                            x2f[b0 + bb, hh, q] += np.float32(dp / w64[cc, tt])
                            xs64[b0 + bb, hh, q] = (x0[b0 + bb, hh, q].astype(np.float64)
                                                    + x1[b0 + bb, hh, q].astype(np.float64)
                                                    + np.float64(np.float16(x2f[b0 + bb, hh, q])))
                            nfix += 1
                            break
        if nfix == 0:
            break
    x2 = x2f.astype(np.float16)
    splits = [x0, x1, x2]

    X1s = []
    for cr in range(N_CORES):
        X1 = np.empty((108, NBT * NBQ * 6 * 64), np.float16)
        for s in range(3):
            xs = splits[s]
            for bh in range(4):
                # b global = cr*BC + bt*BT + bh*NBQ + bq
                bidx = (cr * BC + np.arange(NBT)[:, None] * BT + bh * NBQ
                        + np.arange(NBQ)[None, :]).reshape(-1)   # [NBT*NBQ]
                blk = xs[bidx][:, :, qidx]                       # [nb, 6, 9, 64]
                blk = blk.transpose(2, 0, 1, 3)                  # [9, nb, 6, 64]
                X1[s * 36 + bh * 9: s * 36 + bh * 9 + 9] = blk.reshape(9, -1)
        X1s.append(X1)
    return consts, X1s


def _build_program():
    import concourse.bass as bass
    import concourse.tile as tile
    from concourse import bacc, mybir

    F = mybir.dt.float32
    H = mybir.dt.float16
    BF = mybir.dt.bfloat16
    AO = mybir.AluOpType
    ACT = mybir.ActivationFunctionType.Identity
    RELU = mybir.ActivationFunctionType.Relu

    nc = bacc.Bacc("TRN2", target_bir_lowering=False)

    NCOL1 = NBT * NBQ * 6 * 64
    X1 = nc.dram_tensor("X1", [108, NCOL1], H, kind="ExternalInput")
    T1 = nc.dram_tensor("T1", [108, 128], H, kind="ExternalInput")
    W2 = nc.dram_tensor("W2", [128, 3, 64], BF, kind="ExternalInput")
    W3 = nc.dram_tensor("W3", [128, 3, 128], BF, kind="ExternalInput")
    W4 = nc.dram_tensor("W4", [128, 6, 128], BF, kind="ExternalInput")
    WF = nc.dram_tensor("WF", [128, 16, 10], BF, kind="ExternalInput")
    vecs = {}
    for nm in ["AL1", "BE1", "AL2", "BE2", "AL3", "BE3", "AL4", "BE4",
               "KM3", "TH1", "NB1"]:
        vecs[nm] = nc.dram_tensor(nm, [128, 1], F, kind="ExternalInput")
    TBF = nc.dram_tensor("TBF", [16, 1], F, kind="ExternalInput")
    OUT = nc.dram_tensor("OUT", [BC, 10], F, kind="ExternalOutput")

    from contextlib import ExitStack
    with tile.TileContext(nc) as tc, ExitStack() as es:
        wp = es.enter_context(tc.tile_pool(name="wp", bufs=1))
        xp_ = es.enter_context(tc.tile_pool(name="xp", bufs=2))
        ap1 = es.enter_context(tc.tile_pool(name="ap1", bufs=2))
        ap2 = es.enter_context(tc.tile_pool(name="ap2", bufs=2))
        ap3 = es.enter_context(tc.tile_pool(name="ap3", bufs=2))
        ap4 = es.enter_context(tc.tile_pool(name="ap4", bufs=2))
        stg = es.enter_context(tc.tile_pool(name="stg", bufs=4))
        # one psum pool: chunks of 4 full banks, double-buffered = all 8 banks
        pcp = es.enter_context(tc.tile_pool(name="pcp", bufs=2, space="PSUM"))

        t1t = wp.tile([108, 128], H)
        nc.sync.dma_start(t1t[:], T1[:])
        w2t = wp.tile([128, 3, 64], BF)
        nc.sync.dma_start(w2t[:], W2[:])
        w3t = wp.tile([128, 3, 128], BF)
        nc.sync.dma_start(w3t[:], W3[:])
        w4t = wp.tile([128, 6, 128], BF)
        nc.sync.dma_start(w4t[:], W4[:])
        wft = wp.tile([128, 16, 10], BF)
        nc.sync.dma_start(wft[:], WF[:])
        vt = {}
        for nm, dr in vecs.items():
            vt[nm] = wp.tile([128, 1], F, tag=nm, name=nm.lower())
            nc.sync.dma_start(vt[nm][:], dr[:])
        tbft = wp.tile([16, 1], F)
        nc.sync.dma_start(tbft[:], TBF[:])

        for bt in range(NBT):
            # ---------- L1 ----------
            x1t = xp_.tile([108, NBQ * 6 * 64], H, tag="x1")
            nc.sync.dma_start(
                x1t[:], X1[:, bt * NBQ * 6 * 64:(bt + 1) * NBQ * 6 * 64])
            a1 = ap1.tile([128, NBQ, 6, 34], BF, tag="a1")
            nc.vector.memset(a1[:, :, :, 0:1], 0.0)
            nc.vector.memset(a1[:, :, :, 33:34], 0.0)
            for c in range(4):
                pc = pcp.tile([128, 4, 512], F, tag="pc")
                for b in range(4):
                    bq = c * 4 + b
                    nc.tensor.matmul(pc[:, b, 0:384], t1t[:],
                                     x1t[:, bq * 384:(bq + 1) * 384],
                                     start=True, stop=True)
                w1c = stg.tile([128, 4, 384], BF, tag="w1c")
                nc.scalar.activation(w1c[:], pc[:, :, 0:384], ACT,
                                     bias=vt["BE1"][:], scale=vt["AL1"][:])
                wv = w1c[:].rearrange("p c (h v e) -> p c h v e", v=32, e=2)
                y1 = stg.tile([128, 4, 6, 32], BF, tag="y1")
                nc.vector.scalar_tensor_tensor(
                    y1[:], wv[:, :, :, :, 0], 191.0, wv[:, :, :, :, 1],
                    AO.max, AO.max)
                nc.vector.tensor_scalar(a1[:, c * 4:c * 4 + 4, :, 1:33],
                                        y1[:], 192.0, 1.0,
                                        AO.subtract, AO.min)
            # ---------- L2 ----------
            # Issue order interleaves the 4 row strips (g) so their rhs
            # streams run concurrently on disjoint 32-row PE strips.
            a2 = ap2.tile([128, 4, 8, 6, 34], BF, tag="a2")
            nc.vector.memset(a2[:, :, :, :, 0:1], 0.0)
            nc.vector.memset(a2[:, :, :, :, 33:34], 0.0)
            for ck in range(4):
                pc = pcp.tile([128, 4, 512], F, tag="pc")
                for t in range(3):
                    for ch in range(2):
                        bq0 = ch * 8 + ck * 2
                        for g in range(4):
                            nc.tensor.matmul(
                                pc[64 * ch:64 * ch + 64, g, 0:384],
                                w2t[32 * g:32 * g + 32, t, :],
                                a1[32 * g:32 * g + 32, bq0:bq0 + 2, :, t:t + 32],
                                start=(t == 0), stop=(t == 2),
                                tile_position=(32 * g, 64 * ch))
                w2c = stg.tile([128, 4, 384], BF, tag="w2c")
                nc.scalar.activation(w2c[:], pc[:, :, 0:384], ACT,
                                     bias=vt["BE2"][:], scale=vt["AL2"][:])
                y2 = stg.tile([128, 4, 384], BF, tag="y2")
                nc.vector.tensor_scalar(y2[:], w2c[:], 191.0, 192.0,
                                        AO.max, AO.subtract)
                nc.vector.tensor_scalar(
                    a2[:, :, ck * 2:ck * 2 + 2, :, 1:33],
                    y2[:].rearrange("p c (b h v) -> p c b h v", b=2, h=6),
                    1.0, None, AO.min)
            # ---------- L3 ----------
            # Interleave the two 64-row strips (ch = batch half) so both
            # streams run concurrently; psum tags shared with L2's pool.
            a3 = ap3.tile([128, 4, 16, 6, 16], BF, tag="a3")
            for g in range(4):
                for j in range(2):
                    pc = pcp.tile([128, 4, 512], F, tag="pc")
                    for t in range(3):
                        for ch in range(2):
                            for bpp in range(2):
                                bp = 2 * j + bpp
                                nc.tensor.matmul(
                                    pc[:, 2 * ch + bpp, 0:384],
                                    w3t[64 * ch:64 * ch + 64, t, :],
                                    a2[64 * ch:64 * ch + 64, g,
                                       bp * 2:bp * 2 + 2, :, t:t + 32],
                                    start=(t == 0), stop=(t == 2),
                                    tile_position=(64 * ch, 0))
                    w3c = stg.tile([128, 4, 384], BF, tag="w3c")
                    nc.scalar.activation(w3c[:], pc[:, :, 0:384], ACT,
                                         bias=vt["BE3"][:],
                                         scale=vt["AL3"][:])
                    wv3 = w3c[:].rearrange("p c (b h v e) -> p c b h v e",
                                           b=2, h=6, e=2)
                    for ch in range(2):
                        y3 = stg.tile([128, 2, 2, 6, 16], BF, tag="y3")
                        nc.vector.scalar_tensor_tensor(
                            y3[:], wv3[:, 2 * ch:2 * ch + 2, :, :, :, 0],
                            191.0, wv3[:, 2 * ch:2 * ch + 2, :, :, :, 1],
                            AO.max, AO.max)
                        nc.vector.tensor_scalar(
                            a3[:, g, 8 * ch + 4 * j:8 * ch + 4 * j + 4, :, :],
                            y3[:].rearrange("p q b h v -> p (q b) h v"),
                            192.0, 1.0, AO.subtract, AO.min)
            # ---------- L4 + FC (one psum chunk: banks 0,1 conv, bank 3 fc) --
            a4 = ap4.tile([128, 4, 16, 16], BF, tag="a4")
            pc = pcp.tile([128, 4, 512], F, tag="pc")
            for h in range(6):
                for ck in range(2):
                    nc.tensor.matmul(pc[:, ck, 0:512], w4t[:, h, :],
                                     a3[:, ck * 2:ck * 2 + 2, :, h, :],
                                     start=(h == 0), stop=(h == 5))
            w4c = stg.tile([128, 2, 512], BF, tag="w4c")
            nc.vector.tensor_scalar(w4c[:], pc[:, 0:2, 0:512],
                                    vt["AL4"][:], vt["BE4"][:],
                                    AO.mult, AO.add)
            y4 = stg.tile([128, 2, 512], BF, tag="y4")
            nc.vector.tensor_scalar(y4[:], w4c[:], 191.0, 192.0,
                                    AO.max, AO.subtract)
            nc.vector.tensor_scalar(
                a4[:],
                y4[:].rearrange("p c (g b v) -> p (c g) b v", g=2, b=16),
                1.0, None, AO.min)
            for w in range(16):
                nc.tensor.matmul(pc[0:10, 3, 0:64], wft[:, w, :],
                                 a4[:, :, :, w],
                                 start=(w == 0), stop=(w == 15))
            fo = stg.tile([16, 64], F, tag="fo")
            nc.scalar.activation(fo[0:10, :], pc[0:10, 3, 0:64], ACT,
                                 bias=tbft[0:10, :], scale=1.0)
            nc.sync.dma_start(
                OUT[bt * BT:(bt + 1) * BT, :].rearrange("b o -> o b"),
                fo[0:10, :])

    nc.finalize()
    return nc


_CACHED = {}


def kernel(**inputs):
    from concourse.bass_utils import run_bass_kernel_spmd

    consts, X1s = _build_host_tensors(inputs)
    if "nc" not in _CACHED:
        _CACHED["nc"] = _build_program()
    nc = _CACHED["nc"]

    in_maps = []
    for cr in range(N_CORES):
        m = {k: np.ascontiguousarray(v) for k, v in consts.items()}
        m["X1"] = np.ascontiguousarray(X1s[cr])
        in_maps.append(m)

    res = run_bass_kernel_spmd(nc, in_maps, list(range(N_CORES)))
    out = np.concatenate([res.results[cr]["OUT"] for cr in range(N_CORES)], 0)
    return out.astype(np.float32)

